# revision 18
# baseline (speedup 1.0000x reference)
"""Multi-head attention (B=4, S=2048, D=1024, H=16, Dh=64) on 8 TRN2 NeuronCores.

Sharding: core c handles batch b = c // 2 and head group g = c % 2 (8 heads
each).  Every core computes Q/K/V projections for its batch+heads, the
attention for those heads, and a *partial* output projection (its heads'
slice of Wo).  The host sums the two partials per batch while unsharding —
the tensor-parallel all-reduce on the output, done during gather.

Per-core dataflow (all matmuls bf16 operands, fp32 PSUM accumulation):
  - host supplies X^T [D, S] per input so the contraction dim is always on
    SBUF partitions; no on-device transposes anywhere.
  - Q^T, K^T stored [hk, S] (hk = 8 heads * 64); V stored [t, hk] with an
    extra ones column per head.
  - logits^T[t, f] = (K^T_h).T @ Q^T_h  (K=64; the two heads of an SBUF
    partition-tile run concurrently via PE row tiling).
  - expS = Exp(0.125 * logits^T) on ScalarE (softmax scale folded into the
    activation's free affine; no max subtraction needed: logits ~ N(0,1)).
  - ctx^T/denna = (V_ones).T @ expS accumulated over t: rows 0..63 are the
    unnormalized ctx^T, row 64 is the softmax denominator — for free.
  - normalization deferred: denominators collected into one [16, CW] tile,
    one batched DVE reciprocal, broadcast across partitions with a tiny
    constant selection matmul on the PE, one tensor_mul per chunk.
  - out_part[f, d] accumulated over the four 128-row chunks of ctx^T.
"""

import sys

sys.path.insert(0, "/opt/trn_rl_repo")

import numpy as np
import ml_dtypes

BF = ml_dtypes.bfloat16

# Problem geometry (hardcoded; the harness always calls with these shapes).
B, S, D, H, Dh = 4, 2048, 1024, 16, 64
N_CORES = 8
H_LOC = H // 2          # heads per core
HK = H_LOC * Dh         # 512


class Cfg:
    def __init__(self, S=S, D=D, hloc=H_LOC, Dh=Dh):
        P = 128
        self.S, self.D, self.hloc, self.Dh = S, D, hloc, Dh
        self.P = P
        self.hk = hloc * Dh
        assert self.hk % P == 0 and self.hk <= 512
        self.MJ = self.hk // P        # partition tiles of hk (2 heads each)
        self.J = hloc // 2
        assert self.MJ == self.J
        self.DC = D // P              # contraction chunks for projections
        self.TT = S // P              # t (key) tiles
        self.CW = min(1024, S)        # f-chunk width
        self.NCC = S // self.CW       # f-chunks
        self.NB = self.CW // 512      # PSUM banks per f-chunk
        self.ND = (D + 511) // 512    # 512-wide slices of D
        self.scale = float(Dh) ** -0.5


def make_sel(cfg):
    """sel[r, (j*NCC+cc)*P + p] = 1 where r == (2j + p//64)*NCC + cc.

    Used as matmul lhsT to broadcast reciprocal-denominator rows across the
    64 partitions of each head's ctx^T slice."""
    rows = cfg.hloc * cfg.NCC
    sel = np.zeros((rows, cfg.J * cfg.NCC * cfg.P), np.float32)
    for j in range(cfg.J):
        for cc in range(cfg.NCC):
            base = (j * cfg.NCC + cc) * cfg.P
            for p in range(cfg.P):
                sel[(2 * j + p // 64) * cfg.NCC + cc, base + p] = 1.0
    return sel


def build_nc(cfg):
    import concourse.bass as bass
    import concourse.mybir as mybir
    import concourse.tile as tile
    from concourse import bacc
    from concourse.bass import ds, ts
    from contextlib import ExitStack

    FP32 = mybir.dt.float32
    BF16 = mybir.dt.bfloat16
    EXP = mybir.ActivationFunctionType.Exp

    P, Dh_, hloc = cfg.P, cfg.Dh, cfg.hloc
    S_, D_, hk = cfg.S, cfg.D, cfg.hk
    J, MJ, DC, TT, CW, NCC, NB, ND = (
        cfg.J, cfg.MJ, cfg.DC, cfg.TT, cfg.CW, cfg.NCC, cfg.NB, cfg.ND)
    selrows = hloc * NCC

    nc = bacc.Bacc("TRN2")
    xq = nc.declare_dram_parameter("xq_t", [D_, S_], BF16, isOutput=False)
    xk = nc.declare_dram_parameter("xk_t", [D_, S_], BF16, isOutput=False)
    xv = nc.declare_dram_parameter("xv_t", [D_, S_], BF16, isOutput=False)
    wq = nc.declare_dram_parameter("wq", [D_, hk], BF16, isOutput=False)
    wk = nc.declare_dram_parameter("wk", [D_, hk], BF16, isOutput=False)
    wv = nc.declare_dram_parameter("wv", [D_, hk], BF16, isOutput=False)
    wo = nc.declare_dram_parameter("wo", [hk, D_], BF16, isOutput=False)
    out = nc.declare_dram_parameter("out_part", [S_, D_], FP32, isOutput=True)

    with tile.TileContext(nc) as tc, ExitStack() as ctx:
        singles = ctx.enter_context(tc.tile_pool(name="singles", bufs=1))

        # ---- persistent SBUF tensors -------------------------------------
        wq_sb = singles.tile([P, DC, hk], BF16, tag="wq", name="wq")
        wk_sb = singles.tile([P, DC, hk], BF16, tag="wk", name="wk")
        wv_sb = singles.tile([P, DC, hk], BF16, tag="wv", name="wv")
        wo_sb = singles.tile([P, MJ, D_], BF16, tag="wo", name="wo")
        qT = [singles.tile([P, S_], BF16, tag=f"qT{j}", name=f"qT{j}") for j in range(MJ)]
        kT = [singles.tile([P, S_], BF16, tag=f"kT{j}", name=f"kT{j}") for j in range(MJ)]
        ct = [singles.tile([P, S_], BF16, tag=f"ct{j}", name=f"ct{j}") for j in range(MJ)]
        vt = [singles.tile([P, hloc, Dh_ + 1], BF16, tag=f"vt{m}", name=f"vt{m}")
              for m in range(TT)]

        # wq first: the very first matmul needs only wq + the first xq chunk
        nc.sync.dma_start(out=wq_sb, in_=wq[:, :].rearrange("(a p) n -> p a n", p=P))

        # ---- phase P: projections ----------------------------------------
        with tc.tile_pool(name="xin", bufs=2) as xpool, \
             tc.tile_pool(name="psumP", bufs=2, space="PSUM") as pps:

            def load_xt(src):
                # one DMA per contraction chunk so the first matmuls can
                # start as soon as chunk 0 lands
                xt = xpool.tile([P, DC, S_], BF16, tag="xt", name="xt")
                src_r = src[:, :].rearrange("(a p) s -> p a s", p=P)
                for dc in range(DC):
                    nc.sync.dma_start(out=xt[:, dc, :], in_=src_r[:, dc, :])
                return xt

            def project_T(xt, w_sb, dst):
                # dst[j][hk_row, f] = sum_d w[d, hk_row] * x^T[d, f]
                for j in range(MJ):
                    for cc in range(NCC):
                        ps = pps.tile([P, CW], FP32, tag="psq", name="psq")
                        for dc in range(DC):
                            for nb in range(NB):
                                nc.tensor.matmul(
                                    ps[:, ts(nb, 512)],
                                    lhsT=w_sb[:, dc, ts(j, P)],
                                    rhs=xt[:, dc, ds(cc * CW + nb * 512, 512)],
                                    start=(dc == 0), stop=(dc == DC - 1))
                        nc.vector.tensor_copy(out=dst[j][:, ds(cc * CW, CW)],
                                              in_=ps)

            xt = load_xt(xq)
            nc.sync.dma_start(out=wk_sb,
                              in_=wk[:, :].rearrange("(a p) n -> p a n", p=P))
            project_T(xt, wq_sb, qT)
            xt = load_xt(xk)
            nc.sync.dma_start(out=wv_sb,
                              in_=wv[:, :].rearrange("(a p) n -> p a n", p=P))
            project_T(xt, wk_sb, kT)
            xt = load_xt(xv)
            nc.sync.dma_start(out=wo_sb,
                              in_=wo[:, :].rearrange("(j p) d -> p j d", p=P))
            # V[t, hk] tiles + ones column per head
            for m in range(TT):
                ps = pps.tile([P, hk], FP32, tag="psv", name="psv")
                for dc in range(DC):
                    nc.tensor.matmul(ps, lhsT=xt[:, dc, ts(m, P)],
                                     rhs=wv_sb[:, dc, :],
                                     start=(dc == 0), stop=(dc == DC - 1))
                nc.vector.tensor_copy(
                    out=vt[m][:, :, 0:Dh_],
                    in_=ps.rearrange("p (h k) -> p h k", h=hloc))
                nc.vector.memset(vt[m][:, :, Dh_:Dh_ + 1], 1.0)

        # ---- phase D: attention (+ incremental softmax normalization) ----
        with tc.tile_pool(name="psumL", bufs=1, space="PSUM") as ppl, \
             tc.tile_pool(name="psumC", bufs=1, space="PSUM") as ppc, \
             tc.tile_pool(name="expp", bufs=3) as epool, \
             tc.tile_pool(name="rbc", bufs=2) as rpool, \
             tc.tile_pool(name="stage", bufs=2) as stpool:

            def logits_pair(j, cc, m):
                plA = ppl.tile([P, CW], FP32, tag="plA", name="plA")
                plB = ppl.tile([P, CW], FP32, tag="plB", name="plB")
                for nb in range(NB):
                    nc.tensor.matmul(
                        plA[:, ts(nb, 512)],
                        lhsT=kT[j][0:64, ts(m, P)],
                        rhs=qT[j][0:64, ds(cc * CW + nb * 512, 512)],
                        start=True, stop=True)
                    nc.tensor.matmul(
                        plB[:, ts(nb, 512)],
                        lhsT=kT[j][64:128, ts(m, P)],
                        rhs=qT[j][64:128, ds(cc * CW + nb * 512, 512)],
                        start=True, stop=True)
                return plA, plB

            for j in range(J):
                hA, hB = 2 * j, 2 * j + 1
                for cc in range(NCC):
                    pcA = ppc.tile([Dh_ + 1, CW], FP32, tag="pcA", name="pcA")
                    pcB = ppc.tile([Dh_ + 1, CW], FP32, tag="pcB", name="pcB")
                    pl = logits_pair(j, cc, 0)
                    for m in range(TT):
                        plA, plB = pl
                        eA = epool.tile([P, CW], BF16, tag="eA", name="eA")
                        nc.scalar.activation(out=eA, in_=plA, func=EXP,
                                             scale=cfg.scale)
                        eB = epool.tile([P, CW], BF16, tag="eB", name="eB")
                        nc.scalar.activation(out=eB, in_=plB, func=EXP,
                                             scale=cfg.scale)
                        # software pipeline: next logits go to the PE queue
                        # ahead of this step's ctx matmuls, so the PE can
                        # refill pl the moment exp(m) frees it and ACT never
                        # starves.
                        if m + 1 < TT:
                            pl = logits_pair(j, cc, m + 1)
                        for nb in range(NB):
                            nc.tensor.matmul(
                                pcA[:, ts(nb, 512)],
                                lhsT=vt[m][:, hA, 0:Dh_ + 1],
                                rhs=eA[:, ts(nb, 512)],
                                start=(m == 0), stop=(m == TT - 1))
                            nc.tensor.matmul(
                                pcB[:, ts(nb, 512)],
                                lhsT=vt[m][:, hB, 0:Dh_ + 1],
                                rhs=eB[:, ts(nb, 512)],
                                start=(m == 0), stop=(m == TT - 1))
                    # --- epilogue: softmax normalization fused into the
                    # PSUM drain.  HW constraints (micro-tested):
                    # reciprocal_approx_fast needs base-0 flat 2D APs, and
                    # gpsimd partition_broadcast needs src on partition 0 /
                    # dst starting at partition 0.  So: lane-aligned copy of
                    # the PSUM denominator row, DMA partition-shift to 0,
                    # fast reciprocal, broadcast; head B is normalized
                    # BEFORE its partition-shift DMA so all DVE ops stay
                    # base-0.
                    # Drain pcA/pcB with four quick copies FIRST so the next
                    # block's ctx accumulation isn't blocked behind the slow
                    # reciprocal chain (bufs=1 PSUM accumulators).
                    tmA = stpool.tile([Dh_, CW], BF16, tag="tmA", name="tmA")
                    nc.vector.tensor_copy(out=tmA, in_=pcA[0:Dh_, :])
                    stA = stpool.tile([Dh_ + 1, CW], FP32, tag="stA", name="stA")
                    nc.vector.tensor_copy(out=stA[Dh_:Dh_ + 1, :],
                                          in_=pcA[Dh_:Dh_ + 1, :])
                    tmB = stpool.tile([Dh_, CW], BF16, tag="tmB", name="tmB")
                    nc.vector.tensor_copy(out=tmB, in_=pcB[0:Dh_, :])
                    stB = stpool.tile([Dh_ + 1, CW], FP32, tag="stB", name="stB")
                    nc.vector.tensor_copy(out=stB[Dh_:Dh_ + 1, :],
                                          in_=pcB[Dh_:Dh_ + 1, :])
                    d0A = rpool.tile([1, CW], FP32, tag="d0A", name="d0A")
                    nc.sync.dma_start(out=d0A, in_=stA[Dh_:Dh_ + 1, :])
                    d0B = rpool.tile([1, CW], FP32, tag="d0B", name="d0B")
                    nc.sync.dma_start(out=d0B, in_=stB[Dh_:Dh_ + 1, :])
                    rA = rpool.tile([1, CW], FP32, tag="rA", name="rA")
                    nc.vector.reciprocal_approx_fast(out=rA, in_=d0A)
                    rB = rpool.tile([1, CW], FP32, tag="rB", name="rB")
                    nc.vector.reciprocal_approx_fast(out=rB, in_=d0B)
                    rbA = rpool.tile([Dh_, CW], FP32, tag="rbA", name="rbA")
                    nc.gpsimd.partition_broadcast(rbA, rA, channels=Dh_)
                    rbB = rpool.tile([Dh_, CW], FP32, tag="rbB", name="rbB")
                    nc.gpsimd.partition_broadcast(rbB, rB, channels=Dh_)
                    # head A: normalize straight into ct (partitions 0..63)
                    nc.vector.tensor_mul(out=ct[j][0:64, ds(cc * CW, CW)],
                                         in0=tmA, in1=rbA)
                    # head B: normalize in place, then DMA-shift to
                    # partitions 64..127.
                    nc.vector.tensor_mul(out=tmB, in0=tmB, in1=rbB)
                    nc.sync.dma_start(out=ct[j][64:128, ds(cc * CW, CW)],
                                      in_=tmB)

        # ---- phase E: output projection ----------------------------------
        with tc.tile_pool(name="psumO", bufs=2, space="PSUM") as ppo, \
             tc.tile_pool(name="outb", bufs=3) as obpool:
            ndw = min(512, D_)
            for ft in range(TT):
                po = ppo.tile([P, D_], FP32, tag="po", name="po")
                for j in range(MJ):
                    for nd in range(D_ // ndw):
                        nc.tensor.matmul(
                            po[:, ts(nd, ndw)],
                            lhsT=ct[j][:, ts(ft, P)],
                            rhs=wo_sb[:, j, ts(nd, ndw)],
                            start=(j == 0), stop=(j == MJ - 1))
                ob = obpool.tile([P, D_], FP32, tag="ob", name="ob")
                nc.vector.tensor_copy(out=ob, in_=po)
                nc.sync.dma_start(out=out[ts(ft, P), :], in_=ob)

    nc.compile()
    return nc


def shard_inputs(cfg, query_input, key_input, value_input, Wq, Wk, Wv, Wo):
    """Per-core input maps: core c -> batch c//2, head group c%2."""
    hloc = cfg.hloc
    in_maps = []
    for c in range(N_CORES):
        b, g = c // 2, c % 2
        hs = slice(g * hloc, (g + 1) * hloc)
        in_maps.append({
            "xq_t": np.ascontiguousarray(query_input[b].T).astype(BF),
            "xk_t": np.ascontiguousarray(key_input[b].T).astype(BF),
            "xv_t": np.ascontiguousarray(value_input[b].T).astype(BF),
            "wq": np.ascontiguousarray(Wq[:, hs, :]).reshape(cfg.D, cfg.hk).astype(BF),
            "wk": np.ascontiguousarray(Wk[:, hs, :]).reshape(cfg.D, cfg.hk).astype(BF),
            "wv": np.ascontiguousarray(Wv[:, hs, :]).reshape(cfg.D, cfg.hk).astype(BF),
            "wo": np.ascontiguousarray(Wo[hs]).reshape(cfg.hk, cfg.D).astype(BF),
        })
    return in_maps


_nc_cache = {}


def _get_nc(cfg):
    key = (cfg.S, cfg.D, cfg.hloc, cfg.Dh)
    if key not in _nc_cache:
        _nc_cache[key] = build_nc(cfg)
    return _nc_cache[key]


def run_spmd(inputs, trace=False, trace_cores=None):
    """Run the 8-core SPMD kernel; returns (output [B,S,D] fp32, BassKernelResults)."""
    from concourse.bass_utils import run_bass_kernel_spmd

    cfg = Cfg()
    nc = _get_nc(cfg)
    in_maps = shard_inputs(cfg, **{k: np.asarray(v) for k, v in inputs.items()})
    res = run_bass_kernel_spmd(nc, in_maps, list(range(N_CORES)),
                               trace=trace, trace_cores=trace_cores)
    out = np.empty((B, S, D), np.float32)
    for b in range(B):
        out[b] = res.results[2 * b]["out_part"] + res.results[2 * b + 1]["out_part"]
    return out, res


def kernel(**inputs):
    out, _ = run_spmd(inputs)
    return out


# revision 19
# speedup vs baseline: 1.2754x; 1.2754x over previous
"""Multi-head attention (B=4, S=2048, D=1024, H=16, Dh=64) on 8 TRN2 NeuronCores.

Sharding: core c handles batch b = c // 2 and head group g = c % 2 (8 heads
each).  Every core computes Q/K/V projections for its batch+heads, the
attention for those heads, and a *partial* output projection (its heads'
slice of Wo).  The host sums the two partials per batch while unsharding —
the tensor-parallel all-reduce on the output, done during gather.

Per-core dataflow (all matmuls bf16 operands, fp32 PSUM accumulation):
  - host supplies X^T [D, S] per input so the contraction dim is always on
    SBUF partitions; no on-device transposes anywhere.
  - Q^T, K^T stored [hk, S] (hk = 8 heads * 64); V stored [t, hk] with an
    extra ones column per head.
  - logits^T[t, f] = (K^T_h).T @ Q^T_h  (K=64; the two heads of an SBUF
    partition-tile run concurrently via PE row tiling).
  - expS = Exp(0.125 * logits^T) on ScalarE (softmax scale folded into the
    activation's free affine; no max subtraction needed: logits ~ N(0,1)).
  - ctx^T/denna = (V_ones).T @ expS accumulated over t: rows 0..63 are the
    unnormalized ctx^T, row 64 is the softmax denominator — for free.
  - normalization deferred: denominators collected into one [16, CW] tile,
    one batched DVE reciprocal, broadcast across partitions with a tiny
    constant selection matmul on the PE, one tensor_mul per chunk.
  - out_part[f, d] accumulated over the four 128-row chunks of ctx^T.
"""

import sys

sys.path.insert(0, "/opt/trn_rl_repo")

import numpy as np
import ml_dtypes

BF = ml_dtypes.bfloat16

# Problem geometry (hardcoded; the harness always calls with these shapes).
B, S, D, H, Dh = 4, 2048, 1024, 16, 64
N_CORES = 8
H_LOC = H // 2          # heads per core
HK = H_LOC * Dh         # 512


class Cfg:
    def __init__(self, S=S, D=D, hloc=H_LOC, Dh=Dh):
        P = 128
        self.S, self.D, self.hloc, self.Dh = S, D, hloc, Dh
        self.P = P
        self.hk = hloc * Dh
        assert self.hk % P == 0 and self.hk <= 512
        self.MJ = self.hk // P        # partition tiles of hk (2 heads each)
        self.J = hloc // 2
        assert self.MJ == self.J
        self.DC = D // P              # contraction chunks for projections
        self.TT = S // P              # t (key) tiles
        self.CW = min(1024, S)        # f-chunk width
        self.NCC = S // self.CW       # f-chunks
        self.NB = self.CW // 512      # PSUM banks per f-chunk
        self.ND = (D + 511) // 512    # 512-wide slices of D
        self.scale = float(Dh) ** -0.5


def make_sel(cfg):
    """sel[r, (j*NCC+cc)*P + p] = 1 where r == (2j + p//64)*NCC + cc.

    Used as matmul lhsT to broadcast reciprocal-denominator rows across the
    64 partitions of each head's ctx^T slice."""
    rows = cfg.hloc * cfg.NCC
    sel = np.zeros((rows, cfg.J * cfg.NCC * cfg.P), np.float32)
    for j in range(cfg.J):
        for cc in range(cfg.NCC):
            base = (j * cfg.NCC + cc) * cfg.P
            for p in range(cfg.P):
                sel[(2 * j + p // 64) * cfg.NCC + cc, base + p] = 1.0
    return sel


def build_nc(cfg):
    import concourse.bass as bass
    import concourse.mybir as mybir
    import concourse.tile as tile
    from concourse import bacc
    from concourse.bass import ds, ts
    from contextlib import ExitStack

    FP32 = mybir.dt.float32
    BF16 = mybir.dt.bfloat16
    EXP = mybir.ActivationFunctionType.Exp

    P, Dh_, hloc = cfg.P, cfg.Dh, cfg.hloc
    S_, D_, hk = cfg.S, cfg.D, cfg.hk
    J, MJ, DC, TT, CW, NCC, NB, ND = (
        cfg.J, cfg.MJ, cfg.DC, cfg.TT, cfg.CW, cfg.NCC, cfg.NB, cfg.ND)
    selrows = hloc * NCC

    nc = bacc.Bacc("TRN2")
    xq = nc.declare_dram_parameter("xq_t", [D_, S_], BF16, isOutput=False)
    xk = nc.declare_dram_parameter("xk_t", [D_, S_], BF16, isOutput=False)
    xv = nc.declare_dram_parameter("xv_t", [D_, S_], BF16, isOutput=False)
    wq = nc.declare_dram_parameter("wq", [D_, hk], BF16, isOutput=False)
    wk = nc.declare_dram_parameter("wk", [D_, hk], BF16, isOutput=False)
    wv = nc.declare_dram_parameter("wv", [D_, hk], BF16, isOutput=False)
    wo = nc.declare_dram_parameter("wo", [hk, D_], BF16, isOutput=False)
    out = nc.declare_dram_parameter("out_part", [S_, D_], FP32, isOutput=True)

    with tile.TileContext(nc) as tc, ExitStack() as ctx:
        singles = ctx.enter_context(tc.tile_pool(name="singles", bufs=1))

        # ---- persistent SBUF tensors -------------------------------------
        wq_sb = singles.tile([P, DC, hk], BF16, tag="wq", name="wq")
        wk_sb = singles.tile([P, DC, hk], BF16, tag="wk", name="wk")
        wv_sb = singles.tile([P, DC, hk], BF16, tag="wv", name="wv")
        wo_sb = singles.tile([P, MJ, D_], BF16, tag="wo", name="wo")
        qT = [singles.tile([P, S_], BF16, tag=f"qT{j}", name=f"qT{j}") for j in range(MJ)]
        kT = [singles.tile([P, S_], BF16, tag=f"kT{j}", name=f"kT{j}") for j in range(MJ)]
        ct = [singles.tile([P, S_], BF16, tag=f"ct{j}", name=f"ct{j}") for j in range(MJ)]
        vt = [singles.tile([P, hloc, Dh_ + 1], BF16, tag=f"vt{m}", name=f"vt{m}")
              for m in range(TT)]

        # wq first: the very first matmul needs only wq + the first xq chunk
        nc.sync.dma_start(out=wq_sb, in_=wq[:, :].rearrange("(a p) n -> p a n", p=P))

        # ---- phase P: projections ----------------------------------------
        with tc.tile_pool(name="xin", bufs=2) as xpool, \
             tc.tile_pool(name="psumP", bufs=2, space="PSUM") as pps:

            def load_xt(src):
                # one DMA per contraction chunk so the first matmuls can
                # start as soon as chunk 0 lands
                xt = xpool.tile([P, DC, S_], BF16, tag="xt", name="xt")
                src_r = src[:, :].rearrange("(a p) s -> p a s", p=P)
                for dc in range(DC):
                    nc.sync.dma_start(out=xt[:, dc, :], in_=src_r[:, dc, :])
                return xt

            def project_T(xt, w_sb, dst):
                # dst[j][hk_row, f] = sum_d w[d, hk_row] * x^T[d, f]
                for j in range(MJ):
                    for cc in range(NCC):
                        ps = pps.tile([P, CW], FP32, tag="psq", name="psq")
                        for dc in range(DC):
                            for nb in range(NB):
                                nc.tensor.matmul(
                                    ps[:, ts(nb, 512)],
                                    lhsT=w_sb[:, dc, ts(j, P)],
                                    rhs=xt[:, dc, ds(cc * CW + nb * 512, 512)],
                                    start=(dc == 0), stop=(dc == DC - 1))
                        nc.vector.tensor_copy(out=dst[j][:, ds(cc * CW, CW)],
                                              in_=ps)

            xt = load_xt(xq)
            nc.sync.dma_start(out=wk_sb,
                              in_=wk[:, :].rearrange("(a p) n -> p a n", p=P))
            project_T(xt, wq_sb, qT)
            xt = load_xt(xk)
            nc.sync.dma_start(out=wv_sb,
                              in_=wv[:, :].rearrange("(a p) n -> p a n", p=P))
            project_T(xt, wk_sb, kT)
            xt = load_xt(xv)
            nc.sync.dma_start(out=wo_sb,
                              in_=wo[:, :].rearrange("(j p) d -> p j d", p=P))
            # V[t, hk] tiles + ones column per head
            for m in range(TT):
                ps = pps.tile([P, hk], FP32, tag="psv", name="psv")
                for dc in range(DC):
                    nc.tensor.matmul(ps, lhsT=xt[:, dc, ts(m, P)],
                                     rhs=wv_sb[:, dc, :],
                                     start=(dc == 0), stop=(dc == DC - 1))
                nc.vector.tensor_copy(
                    out=vt[m][:, :, 0:Dh_],
                    in_=ps.rearrange("p (h k) -> p h k", h=hloc))
                nc.vector.memset(vt[m][:, :, Dh_:Dh_ + 1], 1.0)

        # ---- phase D: attention (+ incremental softmax normalization) ----
        with tc.tile_pool(name="psumL", bufs=1, space="PSUM") as ppl, \
             tc.tile_pool(name="psumC", bufs=1, space="PSUM") as ppc, \
             tc.tile_pool(name="expp", bufs=3) as epool, \
             tc.tile_pool(name="rbc", bufs=2) as rpool, \
             tc.tile_pool(name="stage", bufs=2) as stpool:

            def logits_pair(j, cc, m):
                plA = ppl.tile([P, CW], FP32, tag="plA", name="plA")
                plB = ppl.tile([P, CW], FP32, tag="plB", name="plB")
                for nb in range(NB):
                    nc.tensor.matmul(
                        plA[:, ts(nb, 512)],
                        lhsT=kT[j][0:64, ts(m, P)],
                        rhs=qT[j][0:64, ds(cc * CW + nb * 512, 512)],
                        start=True, stop=True)
                    nc.tensor.matmul(
                        plB[:, ts(nb, 512)],
                        lhsT=kT[j][64:128, ts(m, P)],
                        rhs=qT[j][64:128, ds(cc * CW + nb * 512, 512)],
                        start=True, stop=True)
                return plA, plB

            for j in range(J):
                hA, hB = 2 * j, 2 * j + 1
                for cc in range(NCC):
                    pcA = ppc.tile([Dh_ + 1, CW], FP32, tag="pcA", name="pcA")
                    pcB = ppc.tile([Dh_ + 1, CW], FP32, tag="pcB", name="pcB")
                    pl = logits_pair(j, cc, 0)
                    for m in range(TT):
                        plA, plB = pl
                        eA = epool.tile([P, CW], BF16, tag="eA", name="eA")
                        nc.scalar.activation(out=eA, in_=plA, func=EXP,
                                             scale=cfg.scale)
                        eB = epool.tile([P, CW], BF16, tag="eB", name="eB")
                        nc.scalar.activation(out=eB, in_=plB, func=EXP,
                                             scale=cfg.scale)
                        # software pipeline: next logits go to the PE queue
                        # ahead of this step's ctx matmuls, so the PE can
                        # refill pl the moment exp(m) frees it and ACT never
                        # starves.
                        if m + 1 < TT:
                            pl = logits_pair(j, cc, m + 1)
                        for nb in range(NB):
                            nc.tensor.matmul(
                                pcA[:, ts(nb, 512)],
                                lhsT=vt[m][:, hA, 0:Dh_ + 1],
                                rhs=eA[:, ts(nb, 512)],
                                start=(m == 0), stop=(m == TT - 1))
                            nc.tensor.matmul(
                                pcB[:, ts(nb, 512)],
                                lhsT=vt[m][:, hB, 0:Dh_ + 1],
                                rhs=eB[:, ts(nb, 512)],
                                start=(m == 0), stop=(m == TT - 1))
                    # --- epilogue: softmax normalization fused into the
                    # PSUM drain.  HW constraints (micro-tested):
                    # reciprocal_approx_fast needs base-0 flat 2D APs, and
                    # gpsimd partition_broadcast needs src on partition 0 /
                    # dst starting at partition 0.  So: lane-aligned copy of
                    # the PSUM denominator row, DMA partition-shift to 0,
                    # fast reciprocal, broadcast; head B is normalized
                    # BEFORE its partition-shift DMA so all DVE ops stay
                    # base-0.
                    stA = stpool.tile([Dh_ + 1, CW], FP32, tag="stA", name="stA")
                    nc.vector.tensor_copy(out=stA[Dh_:Dh_ + 1, :],
                                          in_=pcA[Dh_:Dh_ + 1, :])
                    stB = stpool.tile([Dh_ + 1, CW], FP32, tag="stB", name="stB")
                    nc.vector.tensor_copy(out=stB[Dh_:Dh_ + 1, :],
                                          in_=pcB[Dh_:Dh_ + 1, :])
                    d0A = rpool.tile([1, CW], FP32, tag="d0A", name="d0A")
                    nc.sync.dma_start(out=d0A, in_=stA[Dh_:Dh_ + 1, :])
                    d0B = rpool.tile([1, CW], FP32, tag="d0B", name="d0B")
                    nc.sync.dma_start(out=d0B, in_=stB[Dh_:Dh_ + 1, :])
                    rA = rpool.tile([1, CW], FP32, tag="rA", name="rA")
                    nc.vector.reciprocal_approx_fast(out=rA, in_=d0A)
                    rB = rpool.tile([1, CW], FP32, tag="rB", name="rB")
                    nc.vector.reciprocal_approx_fast(out=rB, in_=d0B)
                    rbA = rpool.tile([Dh_, CW], FP32, tag="rbA", name="rbA")
                    nc.gpsimd.partition_broadcast(rbA, rA, channels=Dh_)
                    rbB = rpool.tile([Dh_, CW], FP32, tag="rbB", name="rbB")
                    nc.gpsimd.partition_broadcast(rbB, rB, channels=Dh_)
                    # head A: normalize straight into ct (partitions 0..63)
                    nc.vector.tensor_mul(out=ct[j][0:64, ds(cc * CW, CW)],
                                         in0=pcA[0:Dh_, :], in1=rbA)
                    # head B: normalize into a base-0 temp, then DMA-shift to
                    # partitions 64..127.
                    tmB = stpool.tile([Dh_, CW], BF16, tag="tmB", name="tmB")
                    nc.vector.tensor_mul(out=tmB, in0=pcB[0:Dh_, :], in1=rbB)
                    nc.sync.dma_start(out=ct[j][64:128, ds(cc * CW, CW)],
                                      in_=tmB)

        # ---- phase E: output projection ----------------------------------
        with tc.tile_pool(name="psumO", bufs=2, space="PSUM") as ppo, \
             tc.tile_pool(name="outb", bufs=3) as obpool:
            ndw = min(512, D_)
            for ft in range(TT):
                po = ppo.tile([P, D_], FP32, tag="po", name="po")
                for j in range(MJ):
                    for nd in range(D_ // ndw):
                        nc.tensor.matmul(
                            po[:, ts(nd, ndw)],
                            lhsT=ct[j][:, ts(ft, P)],
                            rhs=wo_sb[:, j, ts(nd, ndw)],
                            start=(j == 0), stop=(j == MJ - 1))
                ob = obpool.tile([P, D_], FP32, tag="ob", name="ob")
                nc.vector.tensor_copy(out=ob, in_=po)
                nc.sync.dma_start(out=out[ts(ft, P), :], in_=ob)

    nc.compile()
    return nc


def shard_inputs(cfg, query_input, key_input, value_input, Wq, Wk, Wv, Wo):
    """Per-core input maps: core c -> batch c//2, head group c%2."""
    hloc = cfg.hloc
    in_maps = []
    for c in range(N_CORES):
        b, g = c // 2, c % 2
        hs = slice(g * hloc, (g + 1) * hloc)
        in_maps.append({
            "xq_t": np.ascontiguousarray(query_input[b].T).astype(BF),
            "xk_t": np.ascontiguousarray(key_input[b].T).astype(BF),
            "xv_t": np.ascontiguousarray(value_input[b].T).astype(BF),
            "wq": np.ascontiguousarray(Wq[:, hs, :]).reshape(cfg.D, cfg.hk).astype(BF),
            "wk": np.ascontiguousarray(Wk[:, hs, :]).reshape(cfg.D, cfg.hk).astype(BF),
            "wv": np.ascontiguousarray(Wv[:, hs, :]).reshape(cfg.D, cfg.hk).astype(BF),
            "wo": np.ascontiguousarray(Wo[hs]).reshape(cfg.hk, cfg.D).astype(BF),
        })
    return in_maps


_nc_cache = {}


def _get_nc(cfg):
    key = (cfg.S, cfg.D, cfg.hloc, cfg.Dh)
    if key not in _nc_cache:
        _nc_cache[key] = build_nc(cfg)
    return _nc_cache[key]


def run_spmd(inputs, trace=False, trace_cores=None):
    """Run the 8-core SPMD kernel; returns (output [B,S,D] fp32, BassKernelResults)."""
    from concourse.bass_utils import run_bass_kernel_spmd

    cfg = Cfg()
    nc = _get_nc(cfg)
    in_maps = shard_inputs(cfg, **{k: np.asarray(v) for k, v in inputs.items()})
    res = run_bass_kernel_spmd(nc, in_maps, list(range(N_CORES)),
                               trace=trace, trace_cores=trace_cores)
    out = np.empty((B, S, D), np.float32)
    for b in range(B):
        out[b] = res.results[2 * b]["out_part"] + res.results[2 * b + 1]["out_part"]
    return out, res


def kernel(**inputs):
    out, _ = run_spmd(inputs)
    return out
